# revision 1
# baseline (speedup 1.0000x reference)
"""DGLJTNNDecoder kernel for 8x Trainium2 NeuronCores (Bass/Tile).

Tree-GRU decoder over B=512 chain-trees (N=48 nodes), T=94 DFS steps,
followed by two MLP heads producing (q_loss, p_loss, q_acc, p_acc).

Sharding: data-parallel over trees, 64 trees per core.  The chain
structure makes every gather index step-local, so the scan runs out of
SBUF with no cross-core communication; per-core partial loss/acc sums
are combined on the host.

Key structure per core (64 trees):
  - gather x = emb[wid] via indirect DMA, PE-transpose to [H, node*tree]
  - precompute per-node projections A_z = WzT^T x (+bz), A_h, A_r so the
    sequential scan only does the recurrent half of each GRU matmul
  - the DFS is two *independent* 47-step chains (forward + backtrack);
    they only couple through the h_v output, which is applied as one
    bulk add after both chains finish
  - heads: fused matmul + relu, log-sum-exp / target-gather / argmax
    reductions on-chip; host combines 8x[128,8] partial sums
"""

import sys

if "/opt/trn_rl_repo" not in sys.path:
    sys.path.insert(0, "/opt/trn_rl_repo")

import numpy as np

# Problem constants (fixed by the reference problem definition).
B, N, H, L, V = 512, 48, 256, 64, 800
NC = 8
BC = B // NC            # 64 trees per core
NF = N - 1              # 47 forward steps (= backward steps)
T = 2 * NF              # 94
NODES = N * BC          # 3072 gathered node rows per core
QBLK = NF + 1           # 48 q-head blocks
PBLK = T + 1            # 95 p-head blocks
PROWS = PBLK * BC       # 6080
PPAD = 48 * 128         # 6144 (p rows padded to full 128-row tiles)

BF16 = True             # bf16 matmul operands (psum/loss math stays f32)

_CACHE = {}


def _build(wob_nonzero: bool):
    import concourse.bass as bass
    import concourse.tile as tile
    from concourse import bacc, mybir
    from concourse.masks import make_identity

    f32 = mybir.dt.float32
    i32 = mybir.dt.int32
    wdt = mybir.dt.bfloat16 if BF16 else f32
    AF = mybir.ActivationFunctionType
    ALU = mybir.AluOpType
    AX = mybir.AxisListType

    # Bacc (not raw Bass): its compile pipeline splits multi-sem waits into
    # event-semaphore instructions — walrus codegen only allows 1 wait per
    # DMA instruction.
    nc = bacc.Bacc()

    def din(name, shape, dtype=f32):
        return nc.declare_dram_parameter(name, list(shape), dtype, isOutput=False)

    # --- DRAM parameters ------------------------------------------------
    gidx = din("gidx", [24, 128], i32)
    tvt = din("tvt", [L, 8 * BC], wdt)  # tree_vec^T replicated 8x along free
    qtgt = din("qtgt", [128, 24])
    ptgt = din("ptgt", [128, 48])
    emb = din("emb", [V, H], wdt)
    WzT = din("WzT", [H, H], wdt); WzB = din("WzB", [H, H], wdt)
    WhT = din("WhT", [H, H], wdt); WhB = din("WhB", [H, H], wdt)
    Wr = din("Wr", [H, H], wdt); Ur = din("Ur", [H, H], wdt)
    UwX = din("UwX", [H, H], wdt); UwH = din("UwH", [H, H], wdt)
    UwL = din("UwL", [L, H], wdt)
    WwH = din("WwH", [H, H], wdt); WwL = din("WwL", [L, H], wdt)
    Wo = din("Wo", [H, V], wdt); Us = din("Us", [H, 1], wdt)
    bz2 = din("bz2", [128, 2]); bh2 = din("bh2", [128, 2]); br2 = din("br2", [128, 2])
    ub2 = din("ub2", [128, 2]); wb2 = din("wb2", [128, 2])
    usb = din("usb", [128, 1])
    wob = din("wob", [1, V]) if wob_nonzero else None
    outp = nc.declare_dram_parameter("outp", [128, 8], f32, isOutput=True)

    def rearr2(ap):
        # DRAM [256, M] -> SBUF [128, 2, M] (kt-major halves of contraction)
        return ap.rearrange("(k p) m -> p k m", p=128)

    with tile.TileContext(nc) as tc:
        with (
            tc.tile_pool(name="persist", bufs=1) as pp,
            tc.tile_pool(name="small", bufs=1) as sp,
        ):
            # --- load weights/constants into SBUF -----------------------
            def loadw(dram, shape, tag, dt=wdt, re2=True):
                t = pp.tile(shape, dt, tag=tag)
                eng = nc.sync if re2 else nc.gpsimd
                eng.dma_start(out=t, in_=rearr2(dram[:]) if re2 else dram[:])
                return t

            wzb_s = loadw(WzB, [128, 2, H], "wzb")
            whb_s = loadw(WhB, [128, 2, H], "whb")
            ur_s = loadw(Ur, [128, 2, H], "ur")
            wzt_s = loadw(WzT, [128, 2, H], "wzt")
            wht_s = loadw(WhT, [128, 2, H], "wht")
            wr_s = loadw(Wr, [128, 2, H], "wr")
            uwx_s = loadw(UwX, [128, 2, H], "uwx")
            uwh_s = loadw(UwH, [128, 2, H], "uwh")
            wwh_s = loadw(WwH, [128, 2, H], "wwh")
            wo_s = loadw(Wo, [128, 2, V], "wo")
            us_s = loadw(Us, [128, 2, 1], "us")
            uwl_s = loadw(UwL, [L, H], "uwl", re2=False)
            wwl_s = loadw(WwL, [L, H], "wwl", re2=False)
            bz_s = loadw(bz2, [128, 2], "bz", dt=f32, re2=False)
            bh_s = loadw(bh2, [128, 2], "bh", dt=f32, re2=False)
            br_s = loadw(br2, [128, 2], "br", dt=f32, re2=False)
            ub_s = loadw(ub2, [128, 2], "ub", dt=f32, re2=False)
            wb_s = loadw(wb2, [128, 2], "wb", dt=f32, re2=False)
            usb_s = loadw(usb, [128, 1], "usb", dt=f32, re2=False)
            qtgt_s = loadw(qtgt, [128, 24], "qtgt", dt=f32, re2=False)
            ptgt_s = loadw(ptgt, [128, 48], "ptgt", dt=f32, re2=False)
            wob_s = loadw(wob, [1, V], "wob", dt=f32, re2=False) if wob_nonzero else None

            idx_s = pp.tile([128, 24], i32, tag="idx")
            nc.gpsimd.dma_start(out=idx_s, in_=gidx[:].rearrange("c p -> p c"))

            # tree-vec replicated 8x along blocks: [64, 8, 64]
            tvrep = pp.tile([L, 8, BC], wdt, tag="tvrep")
            nc.gpsimd.dma_start(out=tvrep, in_=tvt[:].rearrange("l (r b) -> l r b", b=BC))

            ident = pp.tile([128, 128], wdt, tag="ident")
            make_identity(nc, ident)

            iota_f = pp.tile([128, V], f32, tag="iotaf")
            iota_i = pp.tile([128, V], i32, tag="iotai")
            nc.gpsimd.iota(iota_i, pattern=[[1, V]], base=0, channel_multiplier=0)
            nc.vector.tensor_copy(iota_f, iota_i)

            # persistent big tensors
            xt = pp.tile([128, 2, NODES], wdt, tag="xt")        # x^T, node-major
            mfq = pp.tile([128, 2, QBLK, BC], wdt, tag="mfq")   # fwd m_e, slot 0 = 0
            mbw = pp.tile([128, 2, NF, BC], wdt, tag="mbw")     # bwd m_e -> hs_bwd
            nc.vector.memset(mfq[:, :, 0, :], 0.0)

            outp_s = sp.tile([128, 8], f32, tag="outp")
            nc.vector.memset(outp_s, 0.0)
            lse_acc = sp.tile([128, 24], f32, tag="lse")
            qt_acc = sp.tile([128, 24], f32, tag="qta")
            qc_acc = sp.tile([128, 24], f32, tag="qca")

            # --- Phase A: embedding gather + transpose ------------------
            with (
                tc.tile_pool(name="gath", bufs=3) as gp,
                tc.tile_pool(name="tps", bufs=4, space="PSUM") as tpp,
            ):
                for c in range(24):
                    xg = gp.tile([128, H], wdt, tag="xg")
                    nc.gpsimd.indirect_dma_start(
                        out=xg,
                        out_offset=None,
                        in_=emb[:],
                        in_offset=bass.IndirectOffsetOnAxis(ap=idx_s[:, c : c + 1], axis=0),
                    )
                    for ht in range(2):
                        pt = tpp.tile([128, 128], wdt, tag="pt")
                        nc.tensor.transpose(pt, xg[:, ht * 128 : (ht + 1) * 128], ident)
                        nc.scalar.copy(xt[:, ht, c * 128 : (c + 1) * 128], pt)

            # --- Phases B+C under the A-tensor pool ---------------------
            with tc.tile_pool(name="apool", bufs=3) as apool:
                az = apool.tile([128, 2, NODES], wdt, tag="arena")
                ah = apool.tile([128, 2, NODES], wdt, tag="arena")
                ar = apool.tile([128, 2, NODES], wdt, tag="arena")

                # Phase B: per-node projections A_z, A_h, A_r (+ biases)
                with tc.tile_pool(name="prps", bufs=4, space="PSUM") as prps:
                    for w_s, a_t, b_s in (
                        (wzt_s, az, bz_s), (wht_s, ah, bh_s), (wr_s, ar, br_s)
                    ):
                        for mt in range(2):
                            msl = slice(mt * 128, (mt + 1) * 128)
                            for ch in range(6):
                                csl = slice(ch * 512, (ch + 1) * 512)
                                ps = prps.tile([128, 512], f32, tag="ps")
                                for kt in range(2):
                                    nc.tensor.matmul(
                                        ps, w_s[:, kt, msl], xt[:, kt, csl],
                                        start=(kt == 0), stop=(kt == 1),
                                    )
                                nc.scalar.activation(
                                    a_t[:, mt, csl], ps, AF.Identity,
                                    bias=b_s[:, mt : mt + 1],
                                )

                # Phase C: two independent GRU chains
                # fwd: steps t=0..46, src/dst nodes (k, k+1)
                # bwd: steps 47+k,   src/dst nodes (47-k, 46-k)
                with (
                    tc.tile_pool(name="scst", bufs=2) as st,
                    tc.tile_pool(name="scps", bufs=2, space="PSUM") as scps,
                ):
                    rm_prev = {"f": None, "b": None}

                    def gru_step(chn, k):
                        if chn == "f":
                            n_src, n_dst = k, k + 1
                            s_ap = mfq[:, :, k, :]
                            me_ap = mfq[:, :, k + 1, :]
                        else:
                            n_src, n_dst = NF - k, NF - 1 - k
                            s_ap = mfq[:, :, 0, :] if k == 0 else mbw[:, :, k - 1, :]
                            me_ap = mbw[:, :, k, :]
                        rmp = rm_prev[chn]
                        if rmp is None:
                            rmp = mfq[:, :, 0, :]

                        psg = scps.tile([128, 3, 2, BC], f32, tag="ps" + chn)
                        for mt in range(2):
                            msl = slice(mt * 128, (mt + 1) * 128)
                            for kt in range(2):
                                nc.tensor.matmul(
                                    psg[:, 0, mt, :], wzb_s[:, kt, msl], s_ap[:, kt, :],
                                    start=(kt == 0), stop=(kt == 1),
                                )
                        for mt in range(2):
                            msl = slice(mt * 128, (mt + 1) * 128)
                            for kt in range(2):
                                nc.tensor.matmul(
                                    psg[:, 1, mt, :], whb_s[:, kt, msl], rmp[:, kt, :],
                                    start=(kt == 0), stop=(kt == 1),
                                )
                        # z = sigmoid(A_z[src] + WzB^T s)
                        azv = st.tile([128, 2, BC], f32, tag="az" + chn)
                        nc.vector.tensor_add(
                            azv, psg[:, 0], az[:, :, n_src * BC : (n_src + 1) * BC]
                        )
                        zv = st.tile([128, 2, BC], f32, tag="z" + chn)
                        nc.scalar.activation(zv, azv, AF.Sigmoid)
                        # mt = tanh(A_h[src] + WhB^T rm_prev)
                        ahv = st.tile([128, 2, BC], f32, tag="ah" + chn)
                        nc.vector.tensor_add(
                            ahv, psg[:, 1], ah[:, :, n_src * BC : (n_src + 1) * BC]
                        )
                        mtv = st.tile([128, 2, BC], f32, tag="mt" + chn)
                        nc.scalar.activation(mtv, ahv, AF.Tanh)
                        # m_e = s + z*(mt - s)
                        dv = st.tile([128, 2, BC], f32, tag="d" + chn)
                        nc.vector.tensor_sub(dv, mtv, s_ap)
                        zdv = st.tile([128, 2, BC], f32, tag="zd" + chn)
                        nc.vector.tensor_mul(zdv, zv, dv)
                        nc.vector.tensor_add(me_ap, s_ap, zdv)
                        # r = sigmoid(A_r[dst] + Ur^T m_e); rm = r * m_e
                        for mt in range(2):
                            msl = slice(mt * 128, (mt + 1) * 128)
                            for kt in range(2):
                                nc.tensor.matmul(
                                    psg[:, 2, mt, :], ur_s[:, kt, msl], me_ap[:, kt, :],
                                    start=(kt == 0), stop=(kt == 1),
                                )
                        arv = st.tile([128, 2, BC], f32, tag="ar" + chn)
                        nc.vector.tensor_add(
                            arv, psg[:, 2], ar[:, :, n_dst * BC : (n_dst + 1) * BC]
                        )
                        rv = st.tile([128, 2, BC], f32, tag="r" + chn)
                        nc.scalar.activation(rv, arv, AF.Sigmoid)
                        rmv = st.tile([128, 2, BC], wdt, tag="rm" + chn)
                        nc.vector.tensor_mul(rmv, rv, me_ap)
                        rm_prev[chn] = rmv

                    for k in range(NF):
                        gru_step("f", k)
                        gru_step("b", k)

            # --- Phase C2: hs_bwd = m_bwd + m_fwd[reversed] in place ----
            # bwd step k output h_v = me_b(k) + mfq[slot 47-k] for k=0..45.
            # (forward-stride ops only: negative AP strides fault on HW)
            for k in range(46):
                nc.vector.tensor_add(
                    mbw[:, :, k, :], mbw[:, :, k, :], mfq[:, :, 47 - k, :]
                )

            mfq_f = mfq.rearrange("p k s b -> p k (s b)")
            mbw_f = mbw.rearrange("p k s b -> p k (s b)")

            with tc.tile_pool(name="hpool", bufs=3) as hpool:
                # --- Phase D: p-head ------------------------------------
                # p1 = relu(UwX^T x_v + UwH^T hs + UwL^T tv + U_b)
                # p  = Us^T p1 + Us_b
                xta = xt[:]
                # x_v for the backtrack half: nodes 46..0 — materialize the
                # reversed copy (negative AP strides fault on HW)
                xtr = hpool.tile([128, 2, NF, BC], wdt, tag="arena")
                for n in range(NF):
                    nc.scalar.copy(
                        xtr[:, :, 46 - n, :], xt[:, :, n * BC : (n + 1) * BC]
                    )
                xtr_f = xtr.rearrange("p k s b -> p k (s b)")
                p1f = hpool.tile([128, 2, NODES], wdt, tag="arena")
                p1b = hpool.tile([128, 2, NODES], wdt, tag="arena")
                nc.vector.memset(p1b[:, :, NF * BC :], 0.0)

                with tc.tile_pool(name="php", bufs=4, space="PSUM") as php:
                    for part in range(2):
                        for mt in range(2):
                            msl = slice(mt * 128, (mt + 1) * 128)
                            for ch in range(6):
                                c0 = ch * 512
                                cw = 512 if part == 0 else min(512, NF * BC - c0)
                                if cw <= 0:
                                    continue
                                nblk = cw // BC
                                csl = slice(c0, c0 + cw)
                                ps = php.tile([128, 512], f32, tag="php")
                                psv = ps[:, :cw]
                                if part == 0:
                                    rx = xta[:, :, csl]
                                    rh = mfq_f[:, :, csl]
                                else:
                                    rx = xtr_f[:, :, csl]
                                    rh = mbw_f[:, :, csl]
                                for kt in range(2):
                                    nc.tensor.matmul(
                                        psv, uwx_s[:, kt, msl], rx[:, kt],
                                        start=(kt == 0), stop=False,
                                    )
                                for kt in range(2):
                                    nc.tensor.matmul(
                                        psv, uwh_s[:, kt, msl], rh[:, kt],
                                        start=False, stop=False,
                                    )
                                nc.tensor.matmul(
                                    psv, uwl_s[:, msl],
                                    tvrep[:, :nblk, :], start=False, stop=True,
                                )
                                dst = (p1f if part == 0 else p1b)[:, mt, csl]
                                if ch % 2 == 0:
                                    nc.vector.tensor_scalar(
                                        out=dst, in0=psv,
                                        scalar1=ub_s[:, mt : mt + 1], scalar2=0.0,
                                        op0=ALU.add, op1=ALU.max,
                                    )
                                else:
                                    nc.scalar.activation(
                                        dst, psv, AF.Relu, bias=ub_s[:, mt : mt + 1]
                                    )

                    # p2: 48 row-tiles of 128 -> psum [128, 48]
                    psp = php.tile([128, 48], f32, tag="psp")
                    for j in range(48):
                        src = p1f if j < 24 else p1b
                        jj = j if j < 24 else j - 24
                        for kt in range(2):
                            nc.tensor.matmul(
                                psp[:, j : j + 1],
                                src[:, kt, jj * 128 : (jj + 1) * 128],
                                us_s[:, kt, :],
                                start=(kt == 0), stop=(kt == 1),
                            )
                    p_sb = sp.tile([128, 48], f32, tag="psb")
                    nc.scalar.activation(p_sb, psp, AF.Identity, bias=usb_s[:, 0:1])

                # BCE: relu(p) + log1p(exp(-|p|)) - p*tgt; acc: (p>0) == tgt
                # (no softplus ACT table set exists; decompose exactly as
                # the reference does)
                ab_t = sp.tile([128, 48], f32, tag="abt")
                nc.scalar.activation(ab_t, p_sb, AF.Abs)
                en_t = sp.tile([128, 48], f32, tag="ent")
                nc.scalar.activation(en_t, ab_t, AF.Exp, scale=-1.0)
                l1p_t = sp.tile([128, 48], f32, tag="l1p")
                nc.scalar.activation(l1p_t, en_t, AF.Ln, bias=1.0)
                rl_t = sp.tile([128, 48], f32, tag="rlt")
                nc.scalar.activation(rl_t, p_sb, AF.Relu)
                sp_t = sp.tile([128, 48], f32, tag="spt")
                nc.vector.tensor_add(sp_t, l1p_t, rl_t)
                ptt = sp.tile([128, 48], f32, tag="ptt")
                nc.vector.tensor_mul(ptt, p_sb, ptgt_s)
                bce = sp.tile([128, 48], f32, tag="bce")
                nc.vector.tensor_sub(bce, sp_t, ptt)
                nc.vector.reduce_sum(outp_s[:, 0:1], bce, axis=AX.X)
                gtz = sp.tile([128, 48], f32, tag="gtz")
                nc.vector.tensor_scalar(
                    out=gtz, in0=p_sb, scalar1=0.0, scalar2=None, op0=ALU.is_gt
                )
                pcr = sp.tile([128, 48], f32, tag="pcr")
                nc.vector.tensor_tensor(out=pcr, in0=gtz, in1=ptgt_s, op=ALU.is_equal)
                nc.vector.reduce_sum(outp_s[:, 1:2], pcr, axis=AX.X)

                # --- Phase E: q-head ------------------------------------
                q1 = hpool.tile([128, 2, NODES], wdt, tag="arena")
                with tc.tile_pool(name="qhp", bufs=2, space="PSUM") as qhp:
                    for mt in range(2):
                        msl = slice(mt * 128, (mt + 1) * 128)
                        for ch in range(6):
                            csl = slice(ch * 512, (ch + 1) * 512)
                            ps = qhp.tile([128, 512], f32, tag="qps")
                            for kt in range(2):
                                nc.tensor.matmul(
                                    ps, wwh_s[:, kt, msl], mfq_f[:, kt, csl],
                                    start=(kt == 0), stop=False,
                                )
                            nc.tensor.matmul(
                                ps, wwl_s[:, msl], tvrep[:, :8, :],
                                start=False, stop=True,
                            )
                            if ch % 2 == 0:
                                nc.vector.tensor_scalar(
                                    out=q1[:, mt, csl], in0=ps,
                                    scalar1=wb_s[:, mt : mt + 1], scalar2=0.0,
                                    op0=ALU.add, op1=ALU.max,
                                )
                            else:
                                nc.scalar.activation(
                                    q1[:, mt, csl], ps, AF.Relu,
                                    bias=wb_s[:, mt : mt + 1],
                                )

                    # q2 logits per row-tile: [128 rows, 800] in PSUM
                    with tc.tile_pool(name="qsc", bufs=2) as qsc:
                        for j in range(24):
                            psq = qhp.tile([128, V], f32, tag="qlg")
                            for kt in range(2):
                                for n0, nn in ((0, 512), (512, V - 512)):
                                    nc.tensor.matmul(
                                        psq[:, n0 : n0 + nn],
                                        q1[:, kt, j * 128 : (j + 1) * 128],
                                        wo_s[:, kt, n0 : n0 + nn],
                                        start=(kt == 0), stop=(kt == 1),
                                    )
                            if wob_nonzero:
                                wv = wob_s[:]
                                wb_b = bass.AP(
                                    tensor=wv.tensor, offset=wv.offset,
                                    ap=[[0, 128], [1, V]],
                                )
                                nc.vector.tensor_add(psq, psq, wb_b)
                            rmax = qsc.tile([128, 1], f32, tag="rmax")
                            nc.vector.reduce_max(rmax, psq, axis=AX.X)
                            scr = qsc.tile([128, V], f32, tag="scr")
                            sume = qsc.tile([128, 1], f32, tag="sume")
                            nc.scalar.activation(scr, psq, AF.Exp, accum_out=sume)
                            nc.scalar.activation(lse_acc[:, j : j + 1], sume, AF.Ln)
                            nc.vector.scalar_tensor_tensor(
                                out=scr, in0=iota_f, scalar=qtgt_s[:, j : j + 1],
                                in1=psq, op0=ALU.is_equal, op1=ALU.mult,
                                accum_out=qt_acc[:, j : j + 1],
                            )
                            nc.vector.tensor_tensor(
                                out=qc_acc[:, j : j + 1], in0=qt_acc[:, j : j + 1],
                                in1=rmax, op=ALU.is_ge,
                            )

            nc.vector.reduce_sum(outp_s[:, 2:3], lse_acc, axis=AX.X)
            nc.vector.reduce_sum(outp_s[:, 3:4], qt_acc, axis=AX.X)
            nc.vector.reduce_sum(outp_s[:, 4:5], qc_acc, axis=AX.X)
            nc.sync.dma_start(out=outp[:], in_=outp_s)

    # Bacc.finalize runs the compile pipeline (multi-wait splitting into
    # event semaphores, register allocation, nop fusion) — required before
    # walrus sees the BIR; run_bass_via_pjrt does not call it.
    nc.finalize()
    return nc


def _get_nc(wob_nonzero: bool):
    key = ("nc", wob_nonzero, BF16)
    if key not in _CACHE:
        _CACHE[key] = _build(wob_nonzero)
    return _CACHE[key]


def _wdt_np():
    if BF16:
        import ml_dtypes

        return ml_dtypes.bfloat16
    return np.float32


def _prep_inputs(inputs):
    f = lambda k: np.ascontiguousarray(np.asarray(inputs[k]), dtype=np.float32)
    wdt = _wdt_np()
    w = lambda a: np.ascontiguousarray(a).astype(wdt)
    wid = np.asarray(inputs["wid"]).astype(np.int64).reshape(B, N)
    tree_vec = f("tree_vec")
    Wz, bz = f("Wz"), f("bz")
    Wr_, Ur_, br = f("Wr"), f("Ur"), f("br")
    Wh, bh = f("Wh"), f("bh")
    W_w, W_b = f("W_w"), f("W_b")
    U_w, U_b = f("U_w"), f("U_b")
    Wo_w, Wo_b = f("Wo_w"), f("Wo_b")
    Us_w, Us_b = f("Us_w"), f("Us_b")
    emb = f("embedding")

    def c2(v):  # [256] -> [128, 2]
        return np.ascontiguousarray(v.reshape(2, 128).T)

    shared = dict(
        emb=w(emb),
        WzT=w(Wz[:H]), WzB=w(Wz[H:]),
        WhT=w(Wh[:H]), WhB=w(Wh[H:]),
        Wr=w(Wr_), Ur=w(Ur_),
        UwX=w(U_w[:H]), UwH=w(U_w[H : 2 * H]), UwL=w(U_w[2 * H :]),
        WwH=w(W_w[:H]), WwL=w(W_w[H:]),
        Wo=w(Wo_w), Us=w(Us_w),
        bz2=c2(bz), bh2=c2(bh), br2=c2(br), ub2=c2(U_b), wb2=c2(W_b),
        usb=np.full((128, 1), float(Us_b.reshape(-1)[0]), np.float32),
    )
    wob_nonzero = bool(np.any(Wo_b != 0))
    if wob_nonzero:
        shared["wob"] = Wo_b.reshape(1, V)

    # p target pattern: row = i*128 + p -> block t = 2i + p//64; 1.0 for t<=46
    ii, pprt = np.meshgrid(np.arange(48), np.arange(128), indexing="xy")
    tblk = 2 * ii + pprt // 64
    ptgt = np.ascontiguousarray((tblk <= 46).astype(np.float32))

    in_maps = []
    for c in range(NC):
        w2 = wid[c * BC : (c + 1) * BC]          # [64 trees, 48 nodes]
        flat = np.ascontiguousarray(w2.T).reshape(-1)  # order n*64+b
        m = dict(shared)
        m["gidx"] = np.ascontiguousarray(flat.reshape(24, 128)).astype(np.int32)
        m["tvt"] = np.ascontiguousarray(
            np.tile(tree_vec[c * BC : (c + 1) * BC].T, (1, 8))
        ).astype(wdt)
        m["qtgt"] = np.ascontiguousarray(flat.reshape(24, 128).T).astype(np.float32)
        m["ptgt"] = ptgt
        in_maps.append(m)
    return in_maps, wob_nonzero, float(Us_b.reshape(-1)[0])


def _combine(results, us_b):
    S = np.zeros(8, np.float64)
    for r in results:
        S += np.asarray(r["outp"], np.float64).sum(axis=0)
    pad_bce = max(us_b, 0.0) + np.log1p(np.exp(-abs(us_b)))
    pad_corr = 1.0 if us_b <= 0 else 0.0
    n_pad = NC * (PPAD - PROWS)  # 8 * 64
    p_loss = (S[0] - n_pad * pad_bce) / B
    p_acc = (S[1] - n_pad * pad_corr) / (PBLK * B)
    q_loss = (S[2] - S[3]) / B
    q_acc = S[4] / (QBLK * B)
    return np.array([q_loss, p_loss, q_acc, p_acc], np.float32)


def kernel(**inputs) -> np.ndarray:
    from concourse.bass_utils import run_bass_kernel_spmd

    in_maps, wob_nonzero, us_b = _prep_inputs(inputs)
    nc = _get_nc(wob_nonzero)
    res = run_bass_kernel_spmd(nc, in_maps, list(range(NC)))
    return _combine(res.results, us_b)



# revision 6
# speedup vs baseline: 1.3699x; 1.3699x over previous
"""DGLJTNNDecoder kernel for 8x Trainium2 NeuronCores (Bass/Tile), v2.

Tree-GRU decoder over B=512 chain-trees (N=48 nodes), T=94 DFS steps,
followed by two MLP heads producing (q_loss, p_loss, q_acc, p_acc).

Sharding: data-parallel over trees, 64 trees per core; per-core partial
loss/acc sums are combined on the host.

v2 structure (vs the first working version):
  - all sigmoids are computed as scaled tanh: sigma(x) = (1+tanh(x/2))/2
    with the 1/2 input scales folded into host-prescaled weights
    (Wz, bz, Wr, Ur, br).  tanh lives in the exp_and_others ACT table
    set together with exp/relu/identity/abs, so the q-head softmax can
    interleave with the scan without table reloads; only the final Ln
    ops (logsumexp + BCE log1p) need one table switch at the tail.
  - A_z/A_h/A_r (per-node projections, biases folded) are pulled into
    the gate PSUM banks by identity-weight matmuls (start=True) ahead of
    the state matmuls, so the scan has no DVE psum-adds and the tanh
    reads PSUM directly.
  - z and r tanh are fused into one ACT op per chain-step (z(t) and
    r(t-1) both depend only on me(t-1); note dst(t-1) == src(t) on a
    chain, so both prefills read node src(t)).
  - h_v for the p-head is accumulated in *node* order (hbw[n] =
    mbw_slot[47-n] + mfq_slot[n]) which both fixes the off-by-one the
    v1 kernel had (mfq[47-k] instead of mfq[46-k]) and removes the
    reversed-x copy: the p-head backtrack half uses xt directly.
  - head work (phase-B remainder, p-part0/part1 chunks, q1 chunks, q2
    logits+softmax) is emitted between scan iterations through a small
    scheduler that bounds PSUM-pool in-flight units and flushes psum
    evacuations at the start of the following iteration.
"""

import sys

if "/opt/trn_rl_repo" not in sys.path:
    sys.path.insert(0, "/opt/trn_rl_repo")

import numpy as np

# Problem constants (fixed by the reference problem definition).
B, N, H, L, V = 512, 48, 256, 64, 800
NC = 8
BC = B // NC            # 64 trees per core
NF = N - 1              # 47 forward steps (= backward steps)
T = 2 * NF              # 94
NODES = N * BC          # 3072 gathered node rows per core
QBLK = NF + 1           # 48 q-head blocks
PBLK = T + 1            # 95 p-head blocks
PROWS = PBLK * BC       # 6080
PPAD = 48 * 128         # 6144 (p rows padded to full 128-row tiles)

BF16 = True             # bf16 matmul operands (psum/loss math stays f32)

_CACHE = {}


def _build(wob_nonzero: bool):
    import concourse.bass as bass
    import concourse.tile as tile
    from concourse import bacc, mybir
    from concourse.masks import make_identity

    f32 = mybir.dt.float32
    i32 = mybir.dt.int32
    wdt = mybir.dt.bfloat16 if BF16 else f32
    AF = mybir.ActivationFunctionType
    ALU = mybir.AluOpType
    AX = mybir.AxisListType

    nc = bacc.Bacc()

    def din(name, shape, dtype=f32):
        return nc.declare_dram_parameter(name, list(shape), dtype, isOutput=False)

    # --- DRAM parameters (same set as v1; host prescales some weights) --
    gidx = din("gidx", [24, 128], i32)
    tvt = din("tvt", [L, 8 * BC], wdt)  # tree_vec^T replicated 8x along free
    qtgt = din("qtgt", [128, 24])
    ptgt = din("ptgt", [128, 48])
    emb = din("emb", [V, H], wdt)
    WzT = din("WzT", [H, H], wdt); WzB = din("WzB", [H, H], wdt)
    WhT = din("WhT", [H, H], wdt); WhB = din("WhB", [H, H], wdt)
    Wr = din("Wr", [H, H], wdt); Ur = din("Ur", [H, H], wdt)
    UwX = din("UwX", [H, H], wdt); UwH = din("UwH", [H, H], wdt)
    UwL = din("UwL", [L, H], wdt)
    WwH = din("WwH", [H, H], wdt); WwL = din("WwL", [L, H], wdt)
    Wo = din("Wo", [H, V], wdt); Us = din("Us", [H, 1], wdt)
    bz2 = din("bz2", [128, 2]); bh2 = din("bh2", [128, 2]); br2 = din("br2", [128, 2])
    ub2 = din("ub2", [128, 2]); wb2 = din("wb2", [128, 2])
    usb = din("usb", [128, 1])
    wob = din("wob", [1, V]) if wob_nonzero else None
    outp = nc.declare_dram_parameter("outp", [128, 8], f32, isOutput=True)

    def rearr2(ap):
        # DRAM [256, M] -> SBUF [128, 2, M] (kt-major halves of contraction)
        return ap.rearrange("(k p) m -> p k m", p=128)

    with tile.TileContext(nc) as tc:
        with (
            tc.tile_pool(name="persist", bufs=1) as pp,
            tc.tile_pool(name="small", bufs=1) as sp,
        ):
            # --- load weights/constants into SBUF -----------------------
            def loadw(dram, shape, tag, dt=wdt, re2=True):
                t = pp.tile(shape, dt, tag=tag)
                eng = nc.sync if re2 else nc.gpsimd
                eng.dma_start(out=t, in_=rearr2(dram[:]) if re2 else dram[:])
                return t

            wzb_s = loadw(WzB, [128, 2, H], "wzb")
            whb_s = loadw(WhB, [128, 2, H], "whb")
            ur_s = loadw(Ur, [128, 2, H], "ur")
            wzt_s = loadw(WzT, [128, 2, H], "wzt")
            wht_s = loadw(WhT, [128, 2, H], "wht")
            wr_s = loadw(Wr, [128, 2, H], "wr")
            uwx_s = loadw(UwX, [128, 2, H], "uwx")
            uwh_s = loadw(UwH, [128, 2, H], "uwh")
            wwh_s = loadw(WwH, [128, 2, H], "wwh")
            wo_s = loadw(Wo, [128, 2, V], "wo")
            us_s = loadw(Us, [128, 2, 1], "us")
            uwl_s = loadw(UwL, [L, H], "uwl", re2=False)
            wwl_s = loadw(WwL, [L, H], "wwl", re2=False)
            bz_s = loadw(bz2, [128, 2], "bz", dt=f32, re2=False)
            bh_s = loadw(bh2, [128, 2], "bh", dt=f32, re2=False)
            br_s = loadw(br2, [128, 2], "br", dt=f32, re2=False)
            ub_s = loadw(ub2, [128, 2], "ub", dt=f32, re2=False)
            wb_s = loadw(wb2, [128, 2], "wb", dt=f32, re2=False)
            usb_s = loadw(usb, [128, 1], "usb", dt=f32, re2=False)
            qtgt_s = loadw(qtgt, [128, 24], "qtgt", dt=f32, re2=False)
            ptgt_s = loadw(ptgt, [128, 48], "ptgt", dt=f32, re2=False)
            wob_s = loadw(wob, [1, V], "wob", dt=f32, re2=False) if wob_nonzero else None

            idx_s = pp.tile([128, 24], i32, tag="idx")
            nc.gpsimd.dma_start(out=idx_s, in_=gidx[:].rearrange("c p -> p c"))

            # tree-vec replicated 8x along blocks: [64, 8, 64]
            tvrep = pp.tile([L, 8, BC], wdt, tag="tvrep")
            nc.gpsimd.dma_start(out=tvrep, in_=tvt[:].rearrange("l (r b) -> l r b", b=BC))

            ident = pp.tile([128, 128], wdt, tag="ident")
            make_identity(nc, ident)

            iota_f = pp.tile([128, V], f32, tag="iotaf")
            iota_i = pp.tile([128, V], i32, tag="iotai")
            nc.gpsimd.iota(iota_i, pattern=[[1, V]], base=0, channel_multiplier=0)
            nc.vector.tensor_copy(iota_f, iota_i)

            # persistent big tensors
            xt = pp.tile([128, 2, NODES], wdt, tag="xt")        # x^T, node-major
            mfq = pp.tile([128, 2, QBLK, BC], wdt, tag="mfq")   # fwd m, slot0=0
            mbw = pp.tile([128, 2, QBLK, BC], wdt, tag="mbw")   # bwd m, slot0=0
            hbw = pp.tile([128, 2, QBLK, BC], wdt, tag="hbw")   # h_v, node order
            az = pp.tile([128, 2, NODES], wdt, tag="az")
            ah = pp.tile([128, 2, NODES], wdt, tag="ah")
            ar = pp.tile([128, 2, NODES], wdt, tag="ar")
            p0a = pp.tile([128, 2, NODES], wdt, tag="p0a")      # p-head part0 relu
            p1a = pp.tile([128, 2, NODES], wdt, tag="p1a")      # p-head part1 relu
            q1a = pp.tile([128, 2, NODES], wdt, tag="q1a")      # q-head relu
            scr = pp.tile([128, 1024], f32, tag="scr")          # q2 scratch

            nc.vector.memset(mfq[:, :, 0, :], 0.0)
            nc.vector.memset(mbw[:, :, 0, :], 0.0)
            nc.vector.memset(p1a[:, :, NF * BC:], 0.0)          # pad block 47

            outp_s = sp.tile([128, 8], f32, tag="outp")
            nc.vector.memset(outp_s, 0.0)
            sume_acc = sp.tile([128, 24], f32, tag="sume")
            qt_acc = sp.tile([128, 24], f32, tag="qta")
            qc_acc = sp.tile([128, 24], f32, tag="qca")

            mfq_f = mfq.rearrange("p k s b -> p k (s b)")
            hbw_f = hbw.rearrange("p k s b -> p k (s b)")
            xt_v = xt.rearrange("p k (c f) -> p k c f", f=128)

            # --- Phase A: embedding gather + transpose into xt ----------
            with (
                tc.tile_pool(name="gath", bufs=3) as gp,
                tc.tile_pool(name="tps", bufs=2, space="PSUM") as tpp,
            ):
                for c0 in range(0, 24, 2):
                    pt = tpp.tile([128, 2, 2, 128], wdt, tag="pt")
                    for co in range(2):
                        c = c0 + co
                        xg = gp.tile([128, H], wdt, tag="xg")
                        nc.gpsimd.indirect_dma_start(
                            out=xg,
                            out_offset=None,
                            in_=emb[:],
                            in_offset=bass.IndirectOffsetOnAxis(
                                ap=idx_s[:, c : c + 1], axis=0),
                        )
                        for ht in range(2):
                            nc.tensor.transpose(
                                pt[:, ht, co, :], xg[:, ht * 128 : (ht + 1) * 128],
                                ident)
                    # one evac per 2 gathers; dst viewed as [128,2,2,128]
                    if (c0 // 2) % 2 == 0:
                        nc.scalar.copy(xt_v[:, :, c0 : c0 + 2, :], pt)
                    else:
                        nc.vector.tensor_copy(xt_v[:, :, c0 : c0 + 2, :], pt)

            # ================= scan-era pools ==========================
            # PSUM budget (8 banks): 4 scan + 2 q2-logits + 2 head-pipe.
            with (
                tc.tile_pool(name="zrf", bufs=1, space="PSUM") as zrf_p,
                tc.tile_pool(name="zrb", bufs=1, space="PSUM") as zrb_p,
                tc.tile_pool(name="hf", bufs=1, space="PSUM") as hf_p,
                tc.tile_pool(name="hb", bufs=1, space="PSUM") as hb_p,
                tc.tile_pool(name="qps", bufs=1, space="PSUM") as qps_p,
                tc.tile_pool(name="hps", bufs=2, space="PSUM") as hps_p,
                tc.tile_pool(name="st", bufs=4) as st,
            ):
                # ---- evac engine alternation --------------------------
                eng_flip = [0]

                def evac_relu(dst, ps, bias_ap):
                    eng_flip[0] ^= 1
                    if eng_flip[0]:
                        nc.scalar.activation(dst, ps, AF.Relu, bias=bias_ap)
                    else:
                        nc.vector.tensor_scalar(
                            out=dst, in0=ps, scalar1=bias_ap, scalar2=0.0,
                            op0=ALU.add, op1=ALU.max)

                def evac_ident(dst, ps, bias_ap):
                    eng_flip[0] ^= 1
                    if eng_flip[0]:
                        nc.scalar.activation(dst, ps, AF.Identity, bias=bias_ap)
                    else:
                        nc.vector.tensor_scalar(
                            out=dst, in0=ps, scalar1=bias_ap, scalar2=None,
                            op0=ALU.add)

                # ---- head/projection units ----------------------------
                def b_unit(mat, ch, mt):
                    w_s, a_t, b_s = {
                        "z": (wzt_s, az, bz_s),
                        "h": (wht_s, ah, bh_s),
                        "r": (wr_s, ar, br_s),
                    }[mat]
                    msl = slice(mt * 128, (mt + 1) * 128)
                    csl = slice(ch * 512, (ch + 1) * 512)
                    ps = hps_p.tile([128, 512], f32, tag="hps")
                    for kt in range(2):
                        nc.tensor.matmul(
                            ps, w_s[:, kt, msl], xt[:, kt, csl],
                            start=(kt == 0), stop=(kt == 1))
                    return lambda: evac_ident(a_t[:, mt, csl], ps, b_s[:, mt:mt+1])

                def p0_unit(ch, mt):
                    msl = slice(mt * 128, (mt + 1) * 128)
                    csl = slice(ch * 512, (ch + 1) * 512)
                    ps = hps_p.tile([128, 512], f32, tag="hps")
                    for kt in range(2):
                        nc.tensor.matmul(
                            ps, uwx_s[:, kt, msl], xt[:, kt, csl],
                            start=(kt == 0), stop=False)
                    for kt in range(2):
                        nc.tensor.matmul(
                            ps, uwh_s[:, kt, msl], mfq_f[:, kt, csl],
                            start=False, stop=False)
                    nc.tensor.matmul(
                        ps, uwl_s[:, msl], tvrep[:, :8, :],
                        start=False, stop=True)
                    return lambda: evac_relu(p0a[:, mt, csl], ps, ub_s[:, mt:mt+1])

                def p1_unit(u, mt):
                    n0 = 4 * u
                    nn = min(4, NF - n0)        # last unit: nodes 44..46
                    cw = nn * BC
                    msl = slice(mt * 128, (mt + 1) * 128)
                    csl = slice(n0 * BC, n0 * BC + cw)
                    ps = hps_p.tile([128, 512], f32, tag="hps")
                    psv = ps[:, :cw]
                    for kt in range(2):
                        nc.tensor.matmul(
                            psv, uwx_s[:, kt, msl], xt[:, kt, csl],
                            start=(kt == 0), stop=False)
                    for kt in range(2):
                        nc.tensor.matmul(
                            psv, uwh_s[:, kt, msl], hbw_f[:, kt, csl],
                            start=False, stop=False)
                    nc.tensor.matmul(
                        psv, uwl_s[:, msl], tvrep[:, :nn, :],
                        start=False, stop=True)
                    return lambda: evac_relu(p1a[:, mt, csl], psv, ub_s[:, mt:mt+1])

                def q1_unit(ch, mt):
                    msl = slice(mt * 128, (mt + 1) * 128)
                    csl = slice(ch * 512, (ch + 1) * 512)
                    ps = hps_p.tile([128, 512], f32, tag="hps")
                    for kt in range(2):
                        nc.tensor.matmul(
                            ps, wwh_s[:, kt, msl], mfq_f[:, kt, csl],
                            start=(kt == 0), stop=False)
                    nc.tensor.matmul(
                        ps, wwl_s[:, msl], tvrep[:, :8, :],
                        start=False, stop=True)
                    return lambda: evac_relu(q1a[:, mt, csl], ps, wb_s[:, mt:mt+1])

                def q2_unit(j):
                    psq = qps_p.tile([128, 1024], f32, tag="qlg")
                    jsl = slice(j * 128, (j + 1) * 128)
                    for kt in range(2):
                        for n0, nn in ((0, 512), (512, V - 512)):
                            nc.tensor.matmul(
                                psq[:, n0 : n0 + nn],
                                q1a[:, kt, jsl], wo_s[:, kt, n0 : n0 + nn],
                                start=(kt == 0), stop=(kt == 1))
                    pv = psq[:, :V]
                    if wob_nonzero:
                        wv = wob_s[:]
                        wb_b = bass.AP(
                            tensor=wv.tensor, offset=wv.offset,
                            ap=[[0, 128], [1, V]])
                        nc.vector.tensor_add(pv, pv, wb_b)

                    def softmax():
                        nc.scalar.activation(
                            scr[:, :V], pv, AF.Exp,
                            accum_out=sume_acc[:, j : j + 1])
                        nc.vector.scalar_tensor_tensor(
                            out=scr[:, :V], in0=iota_f,
                            scalar=qtgt_s[:, j : j + 1],
                            in1=pv, op0=ALU.is_equal, op1=ALU.mult,
                            accum_out=qt_acc[:, j : j + 1])
                        rmax = st.tile([128, 1], f32, tag="rmax")
                        nc.vector.reduce_max(rmax, pv, axis=AX.X)
                        nc.vector.tensor_tensor(
                            out=qc_acc[:, j : j + 1], in0=qt_acc[:, j : j + 1],
                            in1=rmax, op=ALU.is_ge)
                    return softmax

                # ---- pump scheduler -----------------------------------
                # pending: (ready, seq, kind, deadline, fn); kind "hps"
                # units share the 2-buffer hps pool (<=2 emitted per
                # iteration, evacs flushed next iteration => never more
                # than 2 in flight); kind "q2" uses the qps pool (<=1).
                pending = []
                seq_ctr = [0]

                def enq(ready, kind, fn, deadline=10**9):
                    pending.append([ready, seq_ctr[0], kind, deadline, fn])
                    seq_ctr[0] += 1

                evacs_next = []
                q1_emitted = {}     # ch -> #mt halves emitted

                def pump(t):
                    for ev in evacs_next:
                        ev()
                    evacs_next.clear()
                    budget = {"hps": 2, "q2": 1}
                    pending.sort(key=lambda u: (u[0], u[1]))
                    for u in list(pending):
                        ready, _, kind, deadline, fn = u
                        if ready > t or budget[kind] == 0:
                            continue
                        assert t <= deadline, f"unit past deadline at iter {t}"
                        budget[kind] -= 1
                        pending.remove(u)
                        evacs_next.append(fn())

                # phase-B: chunks 5 and 0 are needed at scan iter 0 - emit
                # them now, evac immediately (prefix, path not critical).
                for ch in (5, 0):
                    for mat in ("z", "h", "r"):
                        for mt in range(2):
                            b_unit(mat, ch, mt)()
                # remaining B chunks queued with deadlines (first use:
                # bwd hits chunk 4 at iter 8, chunk 3 at 16, chunk 2 at 24;
                # fwd hits chunk 1 at 8, chunk 2 at 16, chunk 3 at 24).
                # the evac of a unit emitted at pump(t) runs at pump(t+1),
                # i.e. after scan_iter(t+1): a chunk first read by
                # scan_iter(dl) must have its unit emitted by pump(dl-2).
                for ch, dl in ((4, 8), (1, 8), (3, 16), (2, 16)):
                    for mat in ("z", "h", "r"):
                        for mt in range(2):
                            enq(0, "hps", lambda m=mat, c=ch, k=mt: b_unit(m, c, k),
                                deadline=dl - 2)
                # p0/q1 chunks: chunk ch needs mfq slots <= 8ch+7, which
                # exist after iter 8ch+6; q2 tiles enqueue when their q1
                # chunk's second half is emitted.
                for ch in range(6):
                    rdy = min(8 * ch + 7, NF)
                    for mt in range(2):
                        enq(rdy, "hps", lambda c=ch, k=mt: p0_unit(c, k))

                        def _q1(c=ch, k=mt):
                            ev = q1_unit(c, k)
                            q1_emitted[c] = q1_emitted.get(c, 0) + 1
                            if q1_emitted[c] == 2:
                                for j in range(4 * c, 4 * c + 4):
                                    enq(cur_t[0] + 1, "q2",
                                        lambda jj=j: q2_unit(jj))
                            return ev
                        enq(rdy, "hps", _q1)
                # p1 units: hbw[n] is added at iter max(46-n, n-1); +1 so
                # the DVE add has been emitted before the unit reads it.
                for u in range(12):
                    n0 = 4 * u
                    nn = min(4, NF - n0)
                    rdy = max(max(NF - 1 - n, n - 1) for n in range(n0, n0 + nn)) + 1
                    for mt in range(2):
                        enq(rdy, "hps", lambda uu=u, k=mt: p1_unit(uu, k))

                # ---- scan state tiles ---------------------------------
                zrsf = pp.tile([128, 2, 2, BC], wdt, tag="zrsf")
                zrsb = pp.tile([128, 2, 2, BC], wdt, tag="zrsb")
                rmf = pp.tile([128, 2, BC], wdt, tag="rmf")
                rmb = pp.tile([128, 2, BC], wdt, tag="rmb")
                zr_sb = {"f": zrsf, "b": zrsb}
                rm_t = {"f": rmf, "b": rmb}

                def scan_iter(t):
                    """One interleaved fwd+bwd GRU iteration.

                    Computes z(t) and r(t-1) from me(t-1) (one fused tanh),
                    rm(t-1), mt(t) = tanh(ah + WhB^T rm(t-1)), and
                    me(t) = s + (0.5 z'+0.5)(mt - s) for both chains.
                    dst(t-1) == src(t) on a chain, so the r prefill reads
                    A_r at node src as well.
                    """
                    ps_h = {}
                    for chn, pool in (("f", hf_p), ("b", hb_p)):
                        src = t if chn == "f" else NF - t
                        ps = pool.tile([128, 2, BC], f32, tag="h" + chn)
                        nc.tensor.matmul(
                            ps, ident, ah[:, :, src * BC : (src + 1) * BC],
                            start=True, stop=(t == 0))
                        ps_h[chn] = ps
                    ps_zr = {}
                    for chn, pool, hist in (("f", zrf_p, mfq), ("b", zrb_p, mbw)):
                        src = t if chn == "f" else NF - t
                        sl = slice(src * BC, (src + 1) * BC)
                        ps = pool.tile([128, 2, 2, BC], f32, tag="zr" + chn)
                        s_ap = hist[:, :, t, :]
                        nc.tensor.matmul(
                            ps[:, 0], ident, az[:, :, sl], start=True, stop=False)
                        for mt in range(2):
                            msl = slice(mt * 128, (mt + 1) * 128)
                            for kt in range(2):
                                nc.tensor.matmul(
                                    ps[:, 0, mt, :], wzb_s[:, kt, msl],
                                    s_ap[:, kt, :], start=False, stop=(kt == 1))
                        if t > 0:
                            nc.tensor.matmul(
                                ps[:, 1], ident, ar[:, :, sl],
                                start=True, stop=False)
                            for mt in range(2):
                                msl = slice(mt * 128, (mt + 1) * 128)
                                for kt in range(2):
                                    nc.tensor.matmul(
                                        ps[:, 1, mt, :], ur_s[:, kt, msl],
                                        s_ap[:, kt, :], start=False,
                                        stop=(kt == 1))
                        ps_zr[chn] = ps
                    # fused tanh for z(t) (+ r(t-1) when it exists)
                    for chn in ("f", "b"):
                        if t > 0:
                            nc.scalar.activation(
                                zr_sb[chn], ps_zr[chn], AF.Tanh)
                        else:
                            nc.scalar.activation(
                                zr_sb[chn][:, 0], ps_zr[chn][:, 0], AF.Tanh)
                    # rm(t-1) = (0.5 r' + 0.5) * me(t-1), then H state mms
                    if t > 0:
                        for chn, hist in (("f", mfq), ("b", mbw)):
                            rh = st.tile([128, 2, BC], wdt, tag="rh" + chn)
                            nc.vector.tensor_scalar(
                                out=rh, in0=zr_sb[chn][:, 1], scalar1=0.5,
                                scalar2=0.5, op0=ALU.mult, op1=ALU.add)
                            nc.vector.tensor_mul(
                                rm_t[chn], rh, hist[:, :, t, :])
                        for chn in ("f", "b"):
                            for mt in range(2):
                                msl = slice(mt * 128, (mt + 1) * 128)
                                for kt in range(2):
                                    nc.tensor.matmul(
                                        ps_h[chn][:, mt, :], whb_s[:, kt, msl],
                                        rm_t[chn][:, kt, :],
                                        start=False, stop=(kt == 1))
                    # me(t) = s + (0.5 z' + 0.5)(mt - s)
                    for chn, hist in (("f", mfq), ("b", mbw)):
                        mt_sb = st.tile([128, 2, BC], wdt, tag="mt" + chn)
                        nc.scalar.activation(mt_sb, ps_h[chn], AF.Tanh)
                        zp = st.tile([128, 2, BC], wdt, tag="zp" + chn)
                        nc.vector.tensor_scalar(
                            out=zp, in0=zr_sb[chn][:, 0], scalar1=0.5,
                            scalar2=0.5, op0=ALU.mult, op1=ALU.add)
                        d = st.tile([128, 2, BC], wdt, tag="d" + chn)
                        nc.vector.tensor_sub(d, mt_sb, hist[:, :, t, :])
                        zpd = st.tile([128, 2, BC], wdt, tag="zpd" + chn)
                        nc.vector.tensor_mul(zpd, zp, d)
                        nc.vector.tensor_add(
                            hist[:, :, t + 1, :], hist[:, :, t, :], zpd)
                    # h_v adds that became ready: hbw[n] = mbw[47-n]+mfq[n]
                    for n in range(NF):
                        if max(NF - 1 - n, n - 1) == t:
                            nc.vector.tensor_add(
                                hbw[:, :, n, :], mbw[:, :, QBLK - 1 - n, :],
                                mfq[:, :, n, :])

                cur_t = [0]
                for t in range(NF):
                    cur_t[0] = t
                    scan_iter(t)
                    pump(t)
                # drain remaining units + evacs
                t = NF
                while pending or evacs_next:
                    cur_t[0] = t
                    pump(t)
                    t += 1

                # ---- tail: p2, BCE, softmax finalization --------------
                psp = qps_p.tile([128, 48], f32, tag="qlg")
                for j in range(48):
                    src = p0a if j < 24 else p1a
                    jj = j if j < 24 else j - 24
                    for kt in range(2):
                        nc.tensor.matmul(
                            psp[:, j : j + 1],
                            src[:, kt, jj * 128 : (jj + 1) * 128],
                            us_s[:, kt, :],
                            start=(kt == 0), stop=(kt == 1))
                p_sb = sp.tile([128, 48], f32, tag="psb")
                nc.scalar.activation(p_sb, psp, AF.Identity, bias=usb_s[:, 0:1])

                # BCE: relu(p) + log1p(exp(-|p|)) - p*tgt
                ab_t = sp.tile([128, 48], f32, tag="abt")
                nc.scalar.activation(ab_t, p_sb, AF.Abs)
                en_t = sp.tile([128, 48], f32, tag="ent")
                nc.scalar.activation(en_t, ab_t, AF.Exp, scale=-1.0)
                rl_t = sp.tile([128, 48], f32, tag="rlt")
                nc.scalar.activation(rl_t, p_sb, AF.Relu)
                # the two Ln ops back-to-back: one table switch at the tail
                l1p_t = sp.tile([128, 48], f32, tag="l1p")
                nc.scalar.activation(l1p_t, en_t, AF.Ln, bias=1.0)
                lse_t = sp.tile([128, 24], f32, tag="lse")
                nc.scalar.activation(lse_t, sume_acc, AF.Ln)

                sp_t = sp.tile([128, 48], f32, tag="spt")
                nc.vector.tensor_add(sp_t, l1p_t, rl_t)
                ptt = sp.tile([128, 48], f32, tag="ptt")
                nc.vector.tensor_mul(ptt, p_sb, ptgt_s)
                bce = sp.tile([128, 48], f32, tag="bce")
                nc.vector.tensor_sub(bce, sp_t, ptt)
                nc.vector.reduce_sum(outp_s[:, 0:1], bce, axis=AX.X)
                gtz = sp.tile([128, 48], f32, tag="gtz")
                nc.vector.tensor_scalar(
                    out=gtz, in0=p_sb, scalar1=0.0, scalar2=None, op0=ALU.is_gt)
                pcr = sp.tile([128, 48], f32, tag="pcr")
                nc.vector.tensor_tensor(out=pcr, in0=gtz, in1=ptgt_s,
                                        op=ALU.is_equal)
                nc.vector.reduce_sum(outp_s[:, 1:2], pcr, axis=AX.X)
                nc.vector.reduce_sum(outp_s[:, 2:3], lse_t, axis=AX.X)
                nc.vector.reduce_sum(outp_s[:, 3:4], qt_acc, axis=AX.X)
                nc.vector.reduce_sum(outp_s[:, 4:5], qc_acc, axis=AX.X)
            nc.sync.dma_start(out=outp[:], in_=outp_s)

    nc.finalize()
    return nc


def _get_nc(wob_nonzero: bool):
    key = ("nc", wob_nonzero, BF16)
    if key not in _CACHE:
        _CACHE[key] = _build(wob_nonzero)
    return _CACHE[key]


def _wdt_np():
    if BF16:
        import ml_dtypes

        return ml_dtypes.bfloat16
    return np.float32


def _prep_inputs(inputs):
    f = lambda k: np.ascontiguousarray(np.asarray(inputs[k]), dtype=np.float32)
    wdt = _wdt_np()
    w = lambda a: np.ascontiguousarray(a).astype(wdt)
    wid = np.asarray(inputs["wid"]).astype(np.int64).reshape(B, N)
    tree_vec = f("tree_vec")
    Wz, bz = f("Wz"), f("bz")
    Wr_, Ur_, br = f("Wr"), f("Ur"), f("br")
    Wh, bh = f("Wh"), f("bh")
    W_w, W_b = f("W_w"), f("W_b")
    U_w, U_b = f("U_w"), f("U_b")
    Wo_w, Wo_b = f("Wo_w"), f("Wo_b")
    Us_w, Us_b = f("Us_w"), f("Us_b")
    emb = f("embedding")

    def c2(v):  # [256] -> [128, 2]
        return np.ascontiguousarray(v.reshape(2, 128).T)

    # sigma(x) = (1+tanh(x/2))/2: z and r gate pre-activations are halved
    # host-side; the (t+1)/2 affine is applied on-chip.
    shared = dict(
        emb=w(emb),
        WzT=w(0.5 * Wz[:H]), WzB=w(0.5 * Wz[H:]),
        WhT=w(Wh[:H]), WhB=w(Wh[H:]),
        Wr=w(0.5 * Wr_), Ur=w(0.5 * Ur_),
        UwX=w(U_w[:H]), UwH=w(U_w[H : 2 * H]), UwL=w(U_w[2 * H :]),
        WwH=w(W_w[:H]), WwL=w(W_w[H:]),
        Wo=w(Wo_w), Us=w(Us_w),
        bz2=c2(0.5 * bz), bh2=c2(bh), br2=c2(0.5 * br),
        ub2=c2(U_b), wb2=c2(W_b),
        usb=np.full((128, 1), float(Us_b.reshape(-1)[0]), np.float32),
    )
    wob_nonzero = bool(np.any(Wo_b != 0))
    if wob_nonzero:
        shared["wob"] = Wo_b.reshape(1, V)

    # p target pattern: row = i*128 + p -> block t = 2i + p//64; 1.0 for t<=46
    ii, pprt = np.meshgrid(np.arange(48), np.arange(128), indexing="xy")
    tblk = 2 * ii + pprt // 64
    ptgt = np.ascontiguousarray((tblk <= 46).astype(np.float32))

    in_maps = []
    for c in range(NC):
        w2 = wid[c * BC : (c + 1) * BC]          # [64 trees, 48 nodes]
        flat = np.ascontiguousarray(w2.T).reshape(-1)  # order n*64+b
        m = dict(shared)
        m["gidx"] = np.ascontiguousarray(flat.reshape(24, 128)).astype(np.int32)
        m["tvt"] = np.ascontiguousarray(
            np.tile(tree_vec[c * BC : (c + 1) * BC].T, (1, 8))
        ).astype(wdt)
        m["qtgt"] = np.ascontiguousarray(flat.reshape(24, 128).T).astype(np.float32)
        m["ptgt"] = ptgt
        in_maps.append(m)
    return in_maps, wob_nonzero, float(Us_b.reshape(-1)[0])


def _combine(results, us_b):
    S = np.zeros(8, np.float64)
    for r in results:
        S += np.asarray(r["outp"], np.float64).sum(axis=0)
    pad_bce = max(us_b, 0.0) + np.log1p(np.exp(-abs(us_b)))
    pad_corr = 1.0 if us_b <= 0 else 0.0
    n_pad = NC * (PPAD - PROWS)  # 8 * 64
    p_loss = (S[0] - n_pad * pad_bce) / B
    p_acc = (S[1] - n_pad * pad_corr) / (PBLK * B)
    q_loss = (S[2] - S[3]) / B
    q_acc = S[4] / (QBLK * B)
    return np.array([q_loss, p_loss, q_acc, p_acc], np.float32)


def kernel(**inputs) -> np.ndarray:
    from concourse.bass_utils import run_bass_kernel_spmd

    in_maps, wob_nonzero, us_b = _prep_inputs(inputs)
    nc = _get_nc(wob_nonzero)
    res = run_bass_kernel_spmd(nc, in_maps, list(range(NC)))
    return _combine(res.results, us_b)
